# revision 31
# baseline (speedup 1.0000x reference)
"""Bahdanau additive attention on 8 Trainium2 NeuronCores.

Problem: B=4, Tq=Tv=256, D=U=512 fp32.
  q = query @ W1; k = value @ W2
  scores[b,t,v] = sum_u scale[u] * tanh(q[b,t,u] + k[b,v,u])  (masked)
  attn = softmax_v(scores); context = attn @ value
Returns (context [B,Tq,D], attn [B,Tq,Tv]) like the reference.

Sharding: pure data-parallel over (batch x Tq-half) = 8 shards, one per
core; softmax reduces only over Tv so no collectives are needed.

Per-core kernel design (the tanh volume, 128t*256v*512u = 16.8M elems,
is the irreducible cost; ScalarE at 1 elem/cycle/lane @1.2GHz gives a
~109us floor):
  - layout [u on partitions, (t,v) on free]
  - DVE builds qk = k[u,v] + q[u,t] via tensor_scalar_add (per-partition
    scalar operand, 2x mode)
  - ScalarE does one big in-place Tanh per [128, 16*256] tile
  - PE reduces over u with M=1 matmuls (lhsT = scale chunk [128,1])
    accumulating score rows [1, 512] into 4-bank PSUM tiles
  - DVE drains PSUM->SBUF staging rows; a small SBUF->SBUF DMA scatters
    [1,2048] onto 8 t-partitions (only DMA can cross partitions)
  - softmax on [128t, 256v] (DVE reduces + ScalarE exp), context via PE
    transpose + matmul.
"""

import numpy as np

B, TQ, TV, D, U = 4, 256, 256, 512, 512
P = 128
NCORES = 8
TL = TQ // 2          # 128 query positions per core
UC = U // P           # 4 u-chunks
DC = D // P           # 4 d-chunks
VC = TV // P          # 2 v-chunks
TBLK = 32             # t's per main-loop block
NBLK = TL // TBLK     # 4 blocks

_CACHE = {}


def _split_excess_waits(nc, mybir, cap=1):
    """walrus codegen in this container caps sync-wait commands per
    instruction; hoist extra waits onto same-engine NoOps placed just
    before (engine streams are in-order, so semantics are unchanged)."""
    for f in nc.m.functions:
        for bb in f.blocks:
            new_insts = []
            for ins in bb.instructions:
                si = getattr(ins, "sync_info", None)
                if si is not None and si.on_wait and len(si.on_wait) > cap:
                    waits = list(si.on_wait)
                    extra, keep = waits[:-cap], waits[-cap:]
                    k = 0
                    while extra:
                        chunk, extra = extra[:cap], extra[cap:]
                        nop = mybir.InstNoOp(
                            name=f"{ins.name}-ws{k}",
                            sync_info=mybir.SyncInfo(on_wait=chunk, on_update=[]),
                            bass_nofuse=True,
                            engine=ins.engine,
                        )
                        new_insts.append(nop)
                        k += 1
                    si.on_wait = keep
                new_insts.append(ins)
            bb.instructions[:] = new_insts


def _build():
    import concourse.bass as bass
    from concourse import mybir
    from concourse.tile import TileContext
    from concourse.masks import make_identity

    fp32 = mybir.dt.float32
    fp16 = mybir.dt.float16
    AF = mybir.ActivationFunctionType
    AX = mybir.AxisListType
    ALU = mybir.AluOpType

    nc = bass.Bass()
    # fp16 inputs feed the projections (4x faster PE streaming, negligible
    # error vs the fp16 qk rounding); fp32 value is kept for the context.
    qT_d = nc.declare_dram_parameter("qT", [D, TL], fp16, isOutput=False)
    vT_d = nc.declare_dram_parameter("vT", [D, TV], fp16, isOutput=False)
    v16_d = nc.declare_dram_parameter("v16", [TV, D], fp16, isOutput=False)
    w1_d = nc.declare_dram_parameter("w1", [D, U], fp16, isOutput=False)
    w2_d = nc.declare_dram_parameter("w2", [D, U], fp16, isOutput=False)
    sc_d = nc.declare_dram_parameter("sc", [P, UC], fp32, isOutput=False)
    am_d = nc.declare_dram_parameter("am", [1, TV], fp32, isOutput=False)
    ctx_d = nc.declare_dram_parameter("ctx", [TL, D], fp32, isOutput=True)
    attn_d = nc.declare_dram_parameter("attn", [TL, TV], fp32, isOutput=True)

    with TileContext(nc) as tc:
        with tc.tile_pool(name="const", bufs=1) as const, \
             tc.tile_pool(name="main", bufs=1) as mainp:

            # no-input-dependency setup goes first on gpsimd so the PE
            # warm-up and transposes are never gated by input DMAs
            warm16 = const.tile([P, P], fp16)
            nc.gpsimd.memset(warm16, 0.0)
            identity16 = const.tile([P, P], fp16)
            make_identity(nc, identity16)
            ones_row = const.tile([1, P], fp32)
            nc.gpsimd.memset(ones_row, 1.0)
            scale_sb = const.tile([P, UC], fp32)
            scale_f16 = const.tile([P, UC], fp16)
            amask_row = const.tile([1, TV], fp32)

            v16_sb = mainp.tile([P, VC, D], fp16)
            # fp16 k-projection [u_chunk, v] and pair-duplicated q-projection
            # [u_chunk, t, 2] — the duplication keeps the broadcast-add
            # tensor_tensor's innermost step at +1 over 16-bit pairs, which
            # is what qualifies it for the DVE 2x packed mode.
            kprojT16 = mainp.tile([P, UC, TV], fp16)
            qpair = mainp.tile([P, UC, TL, 2], fp16)
            amask_st = mainp.tile([P, 2 * TV], fp32)
            scores_sb = mainp.tile([P, TV], fp32)

            # ---------------- prologue: transposes + projections ----------------
            with tc.tile_pool(name="w", bufs=1) as wpool, \
                 tc.tile_pool(name="pr", bufs=1) as prpool, \
                 tc.tile_pool(name="prps", bufs=2, space="PSUM") as prps:

                # PE warm-up: ~3.4us of dependency-free matmuls on scratch
                # data flips the HAM clock gate to 8/8 before the projections.
                warm_ps = prps.tile([P, 64], fp32, tag="warm")
                for _ in range(40):
                    nc.tensor.matmul(warm_ps, warm16, warm16[:, 0:64],
                                     start=True, stop=True)

                # chunked input DMAs spread across engine DGE rings so the
                # loads run in parallel; vT/w2 first (k-projection path).
                # qT/vT arrive pre-transposed from the host.
                w1_sb = wpool.tile([P, DC, U], fp16)
                w2_sb = wpool.tile([P, DC, U], fp16)
                vT = prpool.tile([P, DC, TV], fp16)
                qT = prpool.tile([P, DC, TL], fp16)
                rings = [nc.sync, nc.scalar, nc.gpsimd]
                ri = 0
                def ring():
                    nonlocal ri
                    e = rings[ri % len(rings)]
                    ri += 1
                    return e
                for dc in range(DC):
                    ring().dma_start(out=vT[:, dc, :], in_=vT_d[dc * P:(dc + 1) * P, :])
                    ring().dma_start(out=w2_sb[:, dc, :], in_=w2_d[dc * P:(dc + 1) * P, :])
                    ring().dma_start(out=qT[:, dc, :], in_=qT_d[dc * P:(dc + 1) * P, :])
                    ring().dma_start(out=w1_sb[:, dc, :], in_=w1_d[dc * P:(dc + 1) * P, :])
                for vc in range(VC):
                    ring().dma_start(out=v16_sb[:, vc, :], in_=v16_d[vc * P:(vc + 1) * P, :])
                ring().dma_start(out=scale_sb, in_=sc_d[:])
                ring().dma_start(out=amask_row, in_=am_d[:])
                # fp16 copy of scale for the fp16 reduce matmuls
                nc.vector.tensor_copy(out=scale_f16, in_=scale_sb)

            # k_projT[u,v] = sum_d W2[d,u] * vT[d,v]; q analogous
                qprojT = prpool.tile([P, UC, TL], fp32)
                for uc in range(UC):
                    pps2 = prps.tile([P, TV], fp32, tag="projk")
                    for dc in range(DC):
                        nc.tensor.matmul(
                            pps2,
                            w2_sb[:, dc, uc * P:(uc + 1) * P],
                            vT[:, dc, :],
                            start=(dc == 0), stop=(dc == DC - 1))
                    nc.scalar.copy(out=kprojT16[:, uc, :], in_=pps2)
                    pps = prps.tile([P, TL], fp32, tag="projq")
                    for dc in range(DC):
                        nc.tensor.matmul(
                            pps,
                            w1_sb[:, dc, uc * P:(uc + 1) * P],
                            qT[:, dc, :],
                            start=(dc == 0), stop=(dc == DC - 1))
                    # write each q value twice (fp16 pair) straight from psum
                    for rep in range(2):
                        qsl = qpair[:, uc, :, :]
                        qstr = bass.AP(
                            tensor=qsl.tensor, offset=qsl.offset + rep,
                            ap=[qsl.ap[0], [2, TL]])
                        nc.scalar.copy(out=qstr, in_=pps)

                # mask row broadcast across partitions, tiled twice along
                # free (matches the [tpair, v] layout of the stage rows)
                am_ps = prps.tile([P, 2 * TV], fp32, tag="amps")
                asl = amask_row[:]
                am2 = bass.AP(tensor=asl.tensor, offset=asl.offset,
                              ap=[asl.ap[0], [0, 2], [1, TV]])
                nc.tensor.matmul(am_ps, ones_row, am2, start=True, stop=True)
                nc.scalar.copy(out=amask_st, in_=am_ps)

            # ---------------- main loop: qk add -> tanh -> weighted reduce ------
            with tc.tile_pool(name="qk", bufs=2) as qkpool, \
                 tc.tile_pool(name="st", bufs=4) as stpool, \
                 tc.tile_pool(name="scps", bufs=2, space="PSUM") as scpsp:

                for blk in range(NBLK):
                    t0 = blk * TBLK
                    # one tile holds all 4 u-chunks: [uc, t, v] flattened
                    qk = qkpool.tile([P, UC * TBLK * TV], fp16, tag="qk")
                    # finer segments at the very start (earlier first tanh)
                    # and very end (reduce overlaps the last tanh)
                    if blk == 0:
                        segs = [8, 8, 16]
                    elif blk == NBLK - 1:
                        segs = [16, 8, 8]
                    else:
                        segs = [16, 16]
                    s0 = 0
                    for seg in segs:
                        th0 = t0 + s0
                        # qk[p, uc, t, v] = k16[p, uc, v] + q16[p, uc, t]: one
                        # tensor_tensor per (u-chunk, segment); every AP
                        # traverses (t, vpair, 2) with innermost step +1 on
                        # 16-bit data -> 2x packed mode.
                        for uc in range(UC):
                            ksl = kprojT16[:, uc, :]
                            in0 = bass.AP(
                                tensor=ksl.tensor, offset=ksl.offset,
                                ap=[ksl.ap[0], [0, seg], [2, TV // 2], [1, 2]])
                            qsl = qpair[:, uc, th0:th0 + seg, :]
                            in1 = bass.AP(
                                tensor=qsl.tensor, offset=qsl.offset,
                                ap=[qsl.ap[0], qsl.ap[1], [0, TV // 2], [1, 2]])
                            osl = qk[:, uc * TBLK * TV + s0 * TV:
                                     uc * TBLK * TV + (s0 + seg) * TV]
                            out = bass.AP(
                                tensor=osl.tensor, offset=osl.offset,
                                ap=[osl.ap[0], [TV, seg], [2, TV // 2], [1, 2]])
                            nc.vector.tensor_tensor(
                                out=out, in0=in0, in1=in1, op=ALU.add)
                        # in-place tanh over this segment across all u-chunks
                        qsl_all = qk[:]
                        hap = bass.AP(
                            tensor=qsl_all.tensor,
                            offset=qsl_all.offset + s0 * TV,
                            ap=[qsl_all.ap[0], [TBLK * TV, UC], [1, seg * TV]])
                        nc.scalar.activation(out=hap, in_=hap, func=AF.Tanh)
                        for ph in range(seg // 8):
                            # 4 t-pairs land on psum partitions {0,32,64,96}
                            # of one bank via column-tiled M=1 matmuls (4 run
                            # concurrently in distinct 32-col groups of the PE)
                            sc_ps = scpsp.tile([P, 512], fp32, tag="scps")
                            for g in range(4):
                                c0 = (s0 + ph * 8 + g * 2) * TV
                                for uc in range(UC):
                                    off = uc * TBLK * TV + c0
                                    nc.tensor.matmul(
                                        sc_ps[32 * g:32 * g + 1, :],
                                        scale_f16[:, uc:uc + 1],
                                        qk[:, off:off + 2 * TV],
                                        start=(uc == 0), stop=(uc == UC - 1),
                                        tile_position=(0, 32 * g))
                            stage = stpool.tile([P, 512], fp32, tag="stage")
                            nc.vector.tensor_tensor(
                                out=stage, in0=sc_ps, in1=amask_st, op=ALU.add)
                            ts0 = th0 + ph * 8
                            ssl = stage[:]
                            src = bass.AP(
                                tensor=ssl.tensor, offset=ssl.offset,
                                ap=[[ssl.ap[0][0] * 32, 4], [1, 512]])
                            nc.sync.dma_start(
                                out=scores_sb[ts0:ts0 + 8, :], in_=src)
                        s0 += seg

            # ---------------- softmax + context ---------------------------------
            with tc.tile_pool(name="ep", bufs=1) as eppool, \
                 tc.tile_pool(name="sm", bufs=1) as smpool, \
                 tc.tile_pool(name="epps", bufs=2, space="PSUM") as epps:

                negmax = smpool.tile([P, 1], fp32)
                nc.vector.tensor_reduce(
                    out=negmax, in_=scores_sb, axis=AX.X, op=ALU.max, negate=True)
                attn_sb = smpool.tile([P, TV], fp32)
                ssum = smpool.tile([P, 1], fp32)
                nc.scalar.activation(
                    out=attn_sb, in_=scores_sb, func=AF.Exp, bias=negmax,
                    accum_out=ssum)
                rsum = smpool.tile([P, 1], fp32)
                nc.vector.reciprocal(out=rsum, in_=ssum)
                nc.vector.tensor_scalar_mul(attn_sb, attn_sb, rsum)
                nc.sync.dma_start(out=attn_d[:], in_=attn_sb)

                attn16 = eppool.tile([P, TV], fp16)
                nc.vector.tensor_copy(out=attn16, in_=attn_sb)
                attnT = eppool.tile([P, VC, P], fp16)
                for vc in range(VC):
                    trp = epps.tile([P, P], fp16, tag="trp")
                    nc.tensor.transpose(
                        trp, attn16[:, vc * P:(vc + 1) * P], identity16)
                    nc.any.tensor_copy(out=attnT[:, vc, :], in_=trp)
                ctx_ps = epps.tile([P, D], fp32, tag="ctxps")
                for vc in range(VC):
                    nc.tensor.matmul(
                        ctx_ps,
                        attnT[:, vc, :],
                        v16_sb[:, vc, :],
                        start=(vc == 0), stop=(vc == VC - 1))
                ctx_sb = eppool.tile([P, D], fp32)
                nc.vector.tensor_copy(out=ctx_sb, in_=ctx_ps)
                nc.sync.dma_start(out=ctx_d[:], in_=ctx_sb)

    _split_excess_waits(nc, mybir, cap=1)
    return nc


def _get_nc():
    if "nc" not in _CACHE:
        _CACHE["nc"] = _build()
    return _CACHE["nc"]


def build_in_maps(query, value, mask, W1, W2, scale):
    query = np.asarray(query, dtype=np.float32)
    value = np.asarray(value, dtype=np.float32)
    mask = np.asarray(mask)
    W1 = np.asarray(W1, dtype=np.float32)
    W2 = np.asarray(W2, dtype=np.float32)
    scale = np.asarray(scale, dtype=np.float32)

    sc = np.ascontiguousarray(scale.reshape(UC, P).T)          # [128, 4]
    w1_16 = W1.astype(np.float16)
    w2_16 = W2.astype(np.float16)
    in_maps = []
    for core in range(NCORES):
        b, th = core // 2, core % 2
        amask = ((mask[b].astype(np.float32)) - 1.0) * 1e9
        in_maps.append({
            "qT": np.ascontiguousarray(
                query[b, th * TL:(th + 1) * TL, :].T).astype(np.float16),
            "vT": np.ascontiguousarray(value[b].T).astype(np.float16),
            "v16": np.ascontiguousarray(value[b]).astype(np.float16),
            "w1": w1_16,
            "w2": w2_16,
            "sc": sc,
            "am": np.ascontiguousarray(amask.reshape(1, TV)),
        })
    return in_maps


def kernel(query, value, mask, W1, W2, scale):
    from concourse.bass_utils import run_bass_kernel_spmd

    in_maps = build_in_maps(query, value, mask, W1, W2, scale)
    nc = _get_nc()
    res = run_bass_kernel_spmd(nc, in_maps, list(range(NCORES)))

    context = np.empty((B, TQ, D), dtype=np.float32)
    attn = np.empty((B, TQ, TV), dtype=np.float32)
    for core in range(NCORES):
        b, th = core // 2, core % 2
        context[b, th * TL:(th + 1) * TL, :] = res.results[core]["ctx"]
        attn[b, th * TL:(th + 1) * TL, :] = res.results[core]["attn"]
    return context, attn


# revision 33
# speedup vs baseline: 1.0011x; 1.0011x over previous
"""Bahdanau additive attention on 8 Trainium2 NeuronCores.

Problem: B=4, Tq=Tv=256, D=U=512 fp32.
  q = query @ W1; k = value @ W2
  scores[b,t,v] = sum_u scale[u] * tanh(q[b,t,u] + k[b,v,u])  (masked)
  attn = softmax_v(scores); context = attn @ value
Returns (context [B,Tq,D], attn [B,Tq,Tv]) like the reference.

Sharding: pure data-parallel over (batch x Tq-half) = 8 shards, one per
core; softmax reduces only over Tv so no collectives are needed.

Per-core kernel design (the tanh volume, 128t*256v*512u = 16.8M elems,
is the irreducible cost; ScalarE at 1 elem/cycle/lane @1.2GHz gives a
~109us floor):
  - layout [u on partitions, (t,v) on free]
  - DVE builds qk = k[u,v] + q[u,t] via tensor_scalar_add (per-partition
    scalar operand, 2x mode)
  - ScalarE does one big in-place Tanh per [128, 16*256] tile
  - PE reduces over u with M=1 matmuls (lhsT = scale chunk [128,1])
    accumulating score rows [1, 512] into 4-bank PSUM tiles
  - DVE drains PSUM->SBUF staging rows; a small SBUF->SBUF DMA scatters
    [1,2048] onto 8 t-partitions (only DMA can cross partitions)
  - softmax on [128t, 256v] (DVE reduces + ScalarE exp), context via PE
    transpose + matmul.
"""

import numpy as np

B, TQ, TV, D, U = 4, 256, 256, 512, 512
P = 128
NCORES = 8
TL = TQ // 2          # 128 query positions per core
UC = U // P           # 4 u-chunks
DC = D // P           # 4 d-chunks
VC = TV // P          # 2 v-chunks
TBLK = 32             # t's per main-loop block
NBLK = TL // TBLK     # 4 blocks

_CACHE = {}


def _split_excess_waits(nc, mybir, cap=1):
    """walrus codegen in this container caps sync-wait commands per
    instruction; hoist extra waits onto same-engine NoOps placed just
    before (engine streams are in-order, so semantics are unchanged)."""
    for f in nc.m.functions:
        for bb in f.blocks:
            new_insts = []
            for ins in bb.instructions:
                si = getattr(ins, "sync_info", None)
                if si is not None and si.on_wait and len(si.on_wait) > cap:
                    waits = list(si.on_wait)
                    extra, keep = waits[:-cap], waits[-cap:]
                    k = 0
                    while extra:
                        chunk, extra = extra[:cap], extra[cap:]
                        nop = mybir.InstNoOp(
                            name=f"{ins.name}-ws{k}",
                            sync_info=mybir.SyncInfo(on_wait=chunk, on_update=[]),
                            bass_nofuse=True,
                            engine=ins.engine,
                        )
                        new_insts.append(nop)
                        k += 1
                    si.on_wait = keep
                new_insts.append(ins)
            bb.instructions[:] = new_insts


def _build():
    import concourse.bass as bass
    from concourse import mybir
    from concourse.tile import TileContext
    from concourse.masks import make_identity

    fp32 = mybir.dt.float32
    fp16 = mybir.dt.float16
    AF = mybir.ActivationFunctionType
    AX = mybir.AxisListType
    ALU = mybir.AluOpType

    nc = bass.Bass()
    # fp16 inputs feed the projections (4x faster PE streaming, negligible
    # error vs the fp16 qk rounding); fp32 value is kept for the context.
    qT_d = nc.declare_dram_parameter("qT", [D, TL], fp16, isOutput=False)
    vT_d = nc.declare_dram_parameter("vT", [D, TV], fp16, isOutput=False)
    v16_d = nc.declare_dram_parameter("v16", [TV, D], fp16, isOutput=False)
    w1_d = nc.declare_dram_parameter("w1", [D, U], fp16, isOutput=False)
    w2_d = nc.declare_dram_parameter("w2", [D, U], fp16, isOutput=False)
    sc_d = nc.declare_dram_parameter("sc", [P, UC], fp32, isOutput=False)
    am_d = nc.declare_dram_parameter("am", [1, TV], fp32, isOutput=False)
    ctx_d = nc.declare_dram_parameter("ctx", [TL, D], fp32, isOutput=True)
    attn_d = nc.declare_dram_parameter("attn", [TL, TV], fp32, isOutput=True)

    with TileContext(nc) as tc:
        with tc.tile_pool(name="const", bufs=1) as const, \
             tc.tile_pool(name="main", bufs=1) as mainp:

            # no-input-dependency setup goes first on gpsimd so the PE
            # warm-up and transposes are never gated by input DMAs
            warm16 = const.tile([P, P], fp16)
            nc.gpsimd.memset(warm16, 0.0)
            identity16 = const.tile([P, P], fp16)
            make_identity(nc, identity16)
            ones_row = const.tile([1, P], fp32)
            nc.gpsimd.memset(ones_row, 1.0)
            scale_sb = const.tile([P, UC], fp32)
            scale_f16 = const.tile([P, UC], fp16)
            amask_row = const.tile([1, TV], fp32)

            v16_sb = mainp.tile([P, VC, D], fp16)
            # fp16 k-projection [u_chunk, v] and pair-duplicated q-projection
            # [u_chunk, t, 2] — the duplication keeps the broadcast-add
            # tensor_tensor's innermost step at +1 over 16-bit pairs, which
            # is what qualifies it for the DVE 2x packed mode.
            kprojT16 = mainp.tile([P, UC, TV], fp16)
            qpair = mainp.tile([P, UC, TL, 2], fp16)
            amask_st = mainp.tile([P, 2 * TV], fp32)
            scores_sb = mainp.tile([P, TV], fp32)

            # ---------------- prologue: transposes + projections ----------------
            with tc.tile_pool(name="w", bufs=1) as wpool, \
                 tc.tile_pool(name="pr", bufs=1) as prpool, \
                 tc.tile_pool(name="prps", bufs=2, space="PSUM") as prps:

                # PE warm-up: ~3.4us of dependency-free matmuls on scratch
                # data flips the HAM clock gate to 8/8 before the projections.
                warm_ps = prps.tile([P, 64], fp32, tag="warm")
                for _ in range(40):
                    nc.tensor.matmul(warm_ps, warm16, warm16[:, 0:64],
                                     start=True, stop=True)

                # chunked input DMAs spread across engine DGE rings so the
                # loads run in parallel; vT/w2 first (k-projection path).
                # qT/vT arrive pre-transposed from the host.
                w1_sb = wpool.tile([P, DC, U], fp16)
                w2_sb = wpool.tile([P, DC, U], fp16)
                vT = prpool.tile([P, DC, TV], fp16)
                qT = prpool.tile([P, DC, TL], fp16)
                rings = [nc.sync, nc.scalar, nc.gpsimd]
                ri = 0
                def ring():
                    nonlocal ri
                    e = rings[ri % len(rings)]
                    ri += 1
                    return e
                for dc in range(DC):
                    ring().dma_start(out=vT[:, dc, :], in_=vT_d[dc * P:(dc + 1) * P, :])
                    ring().dma_start(out=w2_sb[:, dc, :], in_=w2_d[dc * P:(dc + 1) * P, :])
                    ring().dma_start(out=qT[:, dc, :], in_=qT_d[dc * P:(dc + 1) * P, :])
                    ring().dma_start(out=w1_sb[:, dc, :], in_=w1_d[dc * P:(dc + 1) * P, :])
                for vc in range(VC):
                    ring().dma_start(out=v16_sb[:, vc, :], in_=v16_d[vc * P:(vc + 1) * P, :])
                ring().dma_start(out=scale_sb, in_=sc_d[:])
                ring().dma_start(out=amask_row, in_=am_d[:])
                # fp16 copy of scale for the fp16 reduce matmuls
                nc.vector.tensor_copy(out=scale_f16, in_=scale_sb)

            # k_projT[u,v] = sum_d W2[d,u] * vT[d,v]; q analogous
                qprojT = prpool.tile([P, UC, TL], fp32)
                for uc in range(UC):
                    pps2 = prps.tile([P, TV], fp32, tag="projk")
                    for dc in range(DC):
                        nc.tensor.matmul(
                            pps2,
                            w2_sb[:, dc, uc * P:(uc + 1) * P],
                            vT[:, dc, :],
                            start=(dc == 0), stop=(dc == DC - 1))
                    nc.scalar.copy(out=kprojT16[:, uc, :], in_=pps2)
                    pps = prps.tile([P, TL], fp32, tag="projq")
                    for dc in range(DC):
                        nc.tensor.matmul(
                            pps,
                            w1_sb[:, dc, uc * P:(uc + 1) * P],
                            qT[:, dc, :],
                            start=(dc == 0), stop=(dc == DC - 1))
                    # write each q value twice (fp16 pair) straight from psum
                    for rep in range(2):
                        qsl = qpair[:, uc, :, :]
                        qstr = bass.AP(
                            tensor=qsl.tensor, offset=qsl.offset + rep,
                            ap=[qsl.ap[0], [2, TL]])
                        nc.scalar.copy(out=qstr, in_=pps)

                # mask row broadcast across partitions, tiled twice along
                # free (matches the [tpair, v] layout of the stage rows)
                am_ps = prps.tile([P, 2 * TV], fp32, tag="amps")
                asl = amask_row[:]
                am2 = bass.AP(tensor=asl.tensor, offset=asl.offset,
                              ap=[asl.ap[0], [0, 2], [1, TV]])
                nc.tensor.matmul(am_ps, ones_row, am2, start=True, stop=True)
                nc.scalar.copy(out=amask_st, in_=am_ps)

            # ---------------- main loop: qk add -> tanh -> weighted reduce ------
            with tc.tile_pool(name="qk", bufs=2) as qkpool, \
                 tc.tile_pool(name="st", bufs=4) as stpool, \
                 tc.tile_pool(name="scps", bufs=6, space="PSUM") as scpsp:

                # flat segment list: finer at the start (earlier first tanh)
                # and end (reduce overlaps last tanh)
                seg_list = []          # (blk, s0, seg_len)
                for blk in range(NBLK):
                    if blk == 0:
                        segs = [8, 8, 16]
                    elif blk == NBLK - 1:
                        segs = [16, 8, 8]
                    else:
                        segs = [16, 16]
                    s0 = 0
                    for seg in segs:
                        seg_list.append((blk, s0, seg))
                        s0 += seg

                qk_tiles = {}
                pending = []           # deferred stage/scatter emissions

                def emit_stage(sc_ps, th0):
                    stage = stpool.tile([P, 512], fp32, tag="stage")
                    nc.vector.tensor_tensor(
                        out=stage, in0=sc_ps, in1=amask_st, op=ALU.add)
                    ssl = stage[:]
                    srcap = bass.AP(
                        tensor=ssl.tensor, offset=ssl.offset,
                        ap=[[ssl.ap[0][0] * 32, 4], [1, 512]])
                    nc.sync.dma_start(out=scores_sb[th0:th0 + 8, :], in_=srcap)

                for blk, s0, seg in seg_list:
                    t0 = blk * TBLK
                    th0 = t0 + s0
                    if s0 == 0:
                        # one tile per block holds all 4 u-chunks [uc, t, v]
                        qk_new = qkpool.tile(
                            [P, UC * TBLK * TV], fp16, tag="qk")
                        qk_tiles[blk] = qk_new
                    qk = qk_tiles[blk]
                    # qk[p, uc, t, v] = k16[p, uc, v] + q16[p, uc, t]: one
                    # tensor_tensor per (u-chunk, segment); every AP traverses
                    # (t, vpair, 2) with innermost step +1 on 16-bit data
                    # -> 2x packed mode.
                    for uc in range(UC):
                        ksl = kprojT16[:, uc, :]
                        in0 = bass.AP(
                            tensor=ksl.tensor, offset=ksl.offset,
                            ap=[ksl.ap[0], [0, seg], [2, TV // 2], [1, 2]])
                        qsl = qpair[:, uc, th0:th0 + seg, :]
                        in1 = bass.AP(
                            tensor=qsl.tensor, offset=qsl.offset,
                            ap=[qsl.ap[0], qsl.ap[1], [0, TV // 2], [1, 2]])
                        osl = qk[:, uc * TBLK * TV + s0 * TV:
                                 uc * TBLK * TV + (s0 + seg) * TV]
                        out = bass.AP(
                            tensor=osl.tensor, offset=osl.offset,
                            ap=[osl.ap[0], [TV, seg], [2, TV // 2], [1, 2]])
                        nc.vector.tensor_tensor(
                            out=out, in0=in0, in1=in1, op=ALU.add)
                    # the previous segment's PSUM drains are emitted AFTER this
                    # segment's adds so the in-order DVE stream never blocks
                    # the next tanh's inputs behind a drain
                    for sc_ps_prev, th_prev in pending:
                        emit_stage(sc_ps_prev, th_prev)
                    pending = []
                    # in-place tanh over this segment across all u-chunks
                    qsl_all = qk[:]
                    hap = bass.AP(
                        tensor=qsl_all.tensor,
                        offset=qsl_all.offset + s0 * TV,
                        ap=[qsl_all.ap[0], [TBLK * TV, UC], [1, seg * TV]])
                    nc.scalar.activation(out=hap, in_=hap, func=AF.Tanh)
                    for ph in range(seg // 8):
                        # 4 t-pairs land on psum partitions {0,32,64,96} of one
                        # bank via column-tiled M=1 matmuls (4 run concurrently
                        # in distinct 32-col groups of the PE array)
                        sc_ps = scpsp.tile([P, 512], fp32, tag="scps")
                        for g in range(4):
                            c0 = (s0 + ph * 8 + g * 2) * TV
                            for uc in range(UC):
                                off = uc * TBLK * TV + c0
                                nc.tensor.matmul(
                                    sc_ps[32 * g:32 * g + 1, :],
                                    scale_f16[:, uc:uc + 1],
                                    qk[:, off:off + 2 * TV],
                                    start=(uc == 0), stop=(uc == UC - 1),
                                    tile_position=(0, 32 * g))
                        pending.append((sc_ps, th0 + ph * 8))
                for sc_ps_prev, th_prev in pending:
                    emit_stage(sc_ps_prev, th_prev)

            # ---------------- softmax + context ---------------------------------
            with tc.tile_pool(name="ep", bufs=1) as eppool, \
                 tc.tile_pool(name="sm", bufs=1) as smpool, \
                 tc.tile_pool(name="epps", bufs=2, space="PSUM") as epps:

                negmax = smpool.tile([P, 1], fp32)
                nc.vector.tensor_reduce(
                    out=negmax, in_=scores_sb, axis=AX.X, op=ALU.max, negate=True)
                attn_sb = smpool.tile([P, TV], fp32)
                ssum = smpool.tile([P, 1], fp32)
                nc.scalar.activation(
                    out=attn_sb, in_=scores_sb, func=AF.Exp, bias=negmax,
                    accum_out=ssum)
                rsum = smpool.tile([P, 1], fp32)
                nc.vector.reciprocal(out=rsum, in_=ssum)
                nc.vector.tensor_scalar_mul(attn_sb, attn_sb, rsum)
                nc.sync.dma_start(out=attn_d[:], in_=attn_sb)

                attn16 = eppool.tile([P, TV], fp16)
                nc.vector.tensor_copy(out=attn16, in_=attn_sb)
                attnT = eppool.tile([P, VC, P], fp16)
                for vc in range(VC):
                    trp = epps.tile([P, P], fp16, tag="trp")
                    nc.tensor.transpose(
                        trp, attn16[:, vc * P:(vc + 1) * P], identity16)
                    nc.any.tensor_copy(out=attnT[:, vc, :], in_=trp)
                ctx_ps = epps.tile([P, D], fp32, tag="ctxps")
                for vc in range(VC):
                    nc.tensor.matmul(
                        ctx_ps,
                        attnT[:, vc, :],
                        v16_sb[:, vc, :],
                        start=(vc == 0), stop=(vc == VC - 1))
                ctx_sb = eppool.tile([P, D], fp32)
                nc.vector.tensor_copy(out=ctx_sb, in_=ctx_ps)
                nc.sync.dma_start(out=ctx_d[:], in_=ctx_sb)

    _split_excess_waits(nc, mybir, cap=1)
    return nc


def _get_nc():
    if "nc" not in _CACHE:
        _CACHE["nc"] = _build()
    return _CACHE["nc"]


def build_in_maps(query, value, mask, W1, W2, scale):
    query = np.asarray(query, dtype=np.float32)
    value = np.asarray(value, dtype=np.float32)
    mask = np.asarray(mask)
    W1 = np.asarray(W1, dtype=np.float32)
    W2 = np.asarray(W2, dtype=np.float32)
    scale = np.asarray(scale, dtype=np.float32)

    sc = np.ascontiguousarray(scale.reshape(UC, P).T)          # [128, 4]
    w1_16 = W1.astype(np.float16)
    w2_16 = W2.astype(np.float16)
    in_maps = []
    for core in range(NCORES):
        b, th = core // 2, core % 2
        amask = ((mask[b].astype(np.float32)) - 1.0) * 1e9
        in_maps.append({
            "qT": np.ascontiguousarray(
                query[b, th * TL:(th + 1) * TL, :].T).astype(np.float16),
            "vT": np.ascontiguousarray(value[b].T).astype(np.float16),
            "v16": np.ascontiguousarray(value[b]).astype(np.float16),
            "w1": w1_16,
            "w2": w2_16,
            "sc": sc,
            "am": np.ascontiguousarray(amask.reshape(1, TV)),
        })
    return in_maps


def kernel(query, value, mask, W1, W2, scale):
    from concourse.bass_utils import run_bass_kernel_spmd

    in_maps = build_in_maps(query, value, mask, W1, W2, scale)
    nc = _get_nc()
    res = run_bass_kernel_spmd(nc, in_maps, list(range(NCORES)))

    context = np.empty((B, TQ, D), dtype=np.float32)
    attn = np.empty((B, TQ, TV), dtype=np.float32)
    for core in range(NCORES):
        b, th = core // 2, core % 2
        context[b, th * TL:(th + 1) * TL, :] = res.results[core]["ctx"]
        attn[b, th * TL:(th + 1) * TL, :] = res.results[core]["attn"]
    return context, attn


# revision 34
# speedup vs baseline: 1.0606x; 1.0594x over previous
"""Bahdanau additive attention on 8 Trainium2 NeuronCores.

Problem: B=4, Tq=Tv=256, D=U=512 fp32.
  q = query @ W1; k = value @ W2
  scores[b,t,v] = sum_u scale[u] * tanh(q[b,t,u] + k[b,v,u])  (masked)
  attn = softmax_v(scores); context = attn @ value
Returns (context [B,Tq,D], attn [B,Tq,Tv]) like the reference.

Sharding: pure data-parallel over (batch x Tq-half) = 8 shards, one per
core; softmax reduces only over Tv so no collectives are needed.

Per-core kernel design (the tanh volume, 128t*256v*512u = 16.8M elems,
is the irreducible cost; ScalarE at 1 elem/cycle/lane @1.2GHz gives a
~109us floor):
  - layout [u on partitions, (t,v) on free]
  - DVE builds qk = k[u,v] + q[u,t] via tensor_scalar_add (per-partition
    scalar operand, 2x mode)
  - ScalarE does one big in-place Tanh per [128, 16*256] tile
  - PE reduces over u with M=1 matmuls (lhsT = scale chunk [128,1])
    accumulating score rows [1, 512] into 4-bank PSUM tiles
  - DVE drains PSUM->SBUF staging rows; a small SBUF->SBUF DMA scatters
    [1,2048] onto 8 t-partitions (only DMA can cross partitions)
  - softmax on [128t, 256v] (DVE reduces + ScalarE exp), context via PE
    transpose + matmul.
"""

import numpy as np

B, TQ, TV, D, U = 4, 256, 256, 512, 512
P = 128
NCORES = 8
TL = TQ // 2          # 128 query positions per core
UC = U // P           # 4 u-chunks
DC = D // P           # 4 d-chunks
VC = TV // P          # 2 v-chunks
TBLK = 32             # t's per main-loop block
NBLK = TL // TBLK     # 4 blocks

_CACHE = {}


def _split_excess_waits(nc, mybir, cap=1):
    """walrus codegen in this container caps sync-wait commands per
    instruction; hoist extra waits onto same-engine NoOps placed just
    before (engine streams are in-order, so semantics are unchanged)."""
    for f in nc.m.functions:
        for bb in f.blocks:
            new_insts = []
            for ins in bb.instructions:
                si = getattr(ins, "sync_info", None)
                if si is not None and si.on_wait and len(si.on_wait) > cap:
                    waits = list(si.on_wait)
                    extra, keep = waits[:-cap], waits[-cap:]
                    k = 0
                    while extra:
                        chunk, extra = extra[:cap], extra[cap:]
                        nop = mybir.InstNoOp(
                            name=f"{ins.name}-ws{k}",
                            sync_info=mybir.SyncInfo(on_wait=chunk, on_update=[]),
                            bass_nofuse=True,
                            engine=ins.engine,
                        )
                        new_insts.append(nop)
                        k += 1
                    si.on_wait = keep
                new_insts.append(ins)
            bb.instructions[:] = new_insts


def _build():
    import concourse.bass as bass
    from concourse import mybir
    from concourse.tile import TileContext
    from concourse.masks import make_identity

    fp32 = mybir.dt.float32
    fp16 = mybir.dt.float16
    AF = mybir.ActivationFunctionType
    AX = mybir.AxisListType
    ALU = mybir.AluOpType

    nc = bass.Bass()
    # fp16 inputs feed the projections (4x faster PE streaming, negligible
    # error vs the fp16 qk rounding); fp32 value is kept for the context.
    qT_d = nc.declare_dram_parameter("qT", [D, TL], fp16, isOutput=False)
    vT_d = nc.declare_dram_parameter("vT", [D, TV], fp16, isOutput=False)
    v16_d = nc.declare_dram_parameter("v16", [TV, D], fp16, isOutput=False)
    w1_d = nc.declare_dram_parameter("w1", [D, U], fp16, isOutput=False)
    w2_d = nc.declare_dram_parameter("w2", [D, U], fp16, isOutput=False)
    sc_d = nc.declare_dram_parameter("sc", [P, UC], fp32, isOutput=False)
    am_d = nc.declare_dram_parameter("am", [1, TV], fp32, isOutput=False)
    ctx_d = nc.declare_dram_parameter("ctx", [TL, D], fp32, isOutput=True)
    attn_d = nc.declare_dram_parameter("attn", [TL, TV], fp32, isOutput=True)

    with TileContext(nc) as tc:
        with tc.tile_pool(name="const", bufs=1) as const, \
             tc.tile_pool(name="main", bufs=1) as mainp:

            # no-input-dependency setup goes first on gpsimd so the PE
            # warm-up and transposes are never gated by input DMAs
            warm16 = const.tile([P, P], fp16)
            nc.gpsimd.memset(warm16, 0.0)
            identity16 = const.tile([P, P], fp16)
            make_identity(nc, identity16)
            ones_row = const.tile([1, P], fp32)
            nc.gpsimd.memset(ones_row, 1.0)
            scale_sb = const.tile([P, UC], fp32)
            scale_f16 = const.tile([P, UC], fp16)
            amask_row = const.tile([1, TV], fp32)

            v16_sb = mainp.tile([P, VC, D], fp16)
            # fp16 k-projection [u_chunk, v] and pair-duplicated q-projection
            # [u_chunk, t, 2] — the duplication keeps the broadcast-add
            # tensor_tensor's innermost step at +1 over 16-bit pairs, which
            # is what qualifies it for the DVE 2x packed mode.
            kprojT16 = mainp.tile([P, UC, TV], fp16)
            qpair = mainp.tile([P, UC, TL, 2], fp16)
            amask_st = mainp.tile([P, 2 * TV], fp32)
            scores_sb = mainp.tile([P, TV], fp32)

            # ---------------- prologue: transposes + projections ----------------
            with tc.tile_pool(name="w", bufs=1) as wpool, \
                 tc.tile_pool(name="pr", bufs=1) as prpool, \
                 tc.tile_pool(name="prps", bufs=2, space="PSUM") as prps:

                # PE warm-up: ~3.4us of dependency-free matmuls on scratch
                # data flips the HAM clock gate to 8/8 before the projections.
                warm_ps = prps.tile([P, 64], fp32, tag="warm")
                for _ in range(40):
                    nc.tensor.matmul(warm_ps, warm16, warm16[:, 0:64],
                                     start=True, stop=True)

                # chunked input DMAs spread across engine DGE rings so the
                # loads run in parallel; vT/w2 first (k-projection path).
                # qT/vT arrive pre-transposed from the host.
                w1_sb = wpool.tile([P, DC, U], fp16)
                w2_sb = wpool.tile([P, DC, U], fp16)
                vT = prpool.tile([P, DC, TV], fp16)
                qT = prpool.tile([P, DC, TL], fp16)
                rings = [nc.sync, nc.scalar, nc.gpsimd]
                ri = 0
                def ring():
                    nonlocal ri
                    e = rings[ri % len(rings)]
                    ri += 1
                    return e
                for dc in range(DC):
                    ring().dma_start(out=vT[:, dc, :], in_=vT_d[dc * P:(dc + 1) * P, :])
                    ring().dma_start(out=w2_sb[:, dc, :], in_=w2_d[dc * P:(dc + 1) * P, :])
                    ring().dma_start(out=qT[:, dc, :], in_=qT_d[dc * P:(dc + 1) * P, :])
                    ring().dma_start(out=w1_sb[:, dc, :], in_=w1_d[dc * P:(dc + 1) * P, :])
                for vc in range(VC):
                    ring().dma_start(out=v16_sb[:, vc, :], in_=v16_d[vc * P:(vc + 1) * P, :])
                ring().dma_start(out=scale_sb, in_=sc_d[:])
                ring().dma_start(out=amask_row, in_=am_d[:])
                # fp16 copy of scale for the fp16 reduce matmuls
                nc.vector.tensor_copy(out=scale_f16, in_=scale_sb)

            # k_projT[u,v] = sum_d W2[d,u] * vT[d,v]; q analogous
                qprojT = prpool.tile([P, UC, TL], fp32)
                for uc in range(UC):
                    pps2 = prps.tile([P, TV], fp32, tag="projk")
                    for dc in range(DC):
                        nc.tensor.matmul(
                            pps2,
                            w2_sb[:, dc, uc * P:(uc + 1) * P],
                            vT[:, dc, :],
                            start=(dc == 0), stop=(dc == DC - 1))
                    nc.scalar.copy(out=kprojT16[:, uc, :], in_=pps2)
                    pps = prps.tile([P, TL], fp32, tag="projq")
                    for dc in range(DC):
                        nc.tensor.matmul(
                            pps,
                            w1_sb[:, dc, uc * P:(uc + 1) * P],
                            qT[:, dc, :],
                            start=(dc == 0), stop=(dc == DC - 1))
                    # write each q value twice (fp16 pair) straight from psum
                    for rep in range(2):
                        qsl = qpair[:, uc, :, :]
                        qstr = bass.AP(
                            tensor=qsl.tensor, offset=qsl.offset + rep,
                            ap=[qsl.ap[0], [2, TL]])
                        nc.scalar.copy(out=qstr, in_=pps)

                # mask row broadcast across partitions, tiled twice along
                # free (matches the [tpair, v] layout of the stage rows)
                am_ps = prps.tile([P, 2 * TV], fp32, tag="amps")
                asl = amask_row[:]
                am2 = bass.AP(tensor=asl.tensor, offset=asl.offset,
                              ap=[asl.ap[0], [0, 2], [1, TV]])
                nc.tensor.matmul(am_ps, ones_row, am2, start=True, stop=True)
                nc.scalar.copy(out=amask_st, in_=am_ps)

            # ---------------- main loop: qk add -> tanh -> weighted reduce ------
            with tc.tile_pool(name="qk", bufs=2) as qkpool, \
                 tc.tile_pool(name="st", bufs=4) as stpool, \
                 tc.tile_pool(name="scps", bufs=6, space="PSUM") as scpsp:

                # flat segment list: finer at the start (earlier first tanh)
                # and end (reduce overlaps last tanh)
                seg_list = []          # (blk, s0, seg_len)
                for blk in range(NBLK):
                    if blk == 0:
                        segs = [8, 8, 16]
                    elif blk == NBLK - 1:
                        segs = [16, 8, 8]
                    else:
                        segs = [16, 16]
                    s0 = 0
                    for seg in segs:
                        seg_list.append((blk, s0, seg))
                        s0 += seg

                qk_tiles = {}
                pending = []           # deferred stage/scatter emissions

                def emit_stage(sc_ps, th0):
                    stage = stpool.tile([P, 512], fp32, tag="stage")
                    nc.vector.tensor_tensor(
                        out=stage, in0=sc_ps, in1=amask_st, op=ALU.add)
                    ssl = stage[:]
                    srcap = bass.AP(
                        tensor=ssl.tensor, offset=ssl.offset,
                        ap=[[ssl.ap[0][0] * 32, 4], [1, 512]])
                    nc.sync.dma_start(out=scores_sb[th0:th0 + 8, :], in_=srcap)

                CELL = UC * TV         # one t-row: 4 u-chunks x 256 v
                for blk, s0, seg in seg_list:
                    t0 = blk * TBLK
                    th0 = t0 + s0
                    if s0 == 0:
                        # layout [t, uc, v]: every segment is one contiguous
                        # byte range, so segment ops have precise disjoint
                        # subtile dependencies
                        qk_new = qkpool.tile(
                            [P, TBLK, UC, TV], fp16, tag="qk")
                        qk_tiles[blk] = qk_new
                    qk = qk_tiles[blk]
                    # qk[p, t, uc, v] = k16[p, uc, v] + q16[p, uc, t]: one
                    # tensor_tensor per (u-chunk, segment); every AP traverses
                    # (t, vpair, 2) with innermost step +1 on 16-bit data
                    # -> 2x packed mode.
                    for uc in range(UC):
                        ksl = kprojT16[:, uc, :]
                        in0 = bass.AP(
                            tensor=ksl.tensor, offset=ksl.offset,
                            ap=[ksl.ap[0], [0, seg], [2, TV // 2], [1, 2]])
                        qsl = qpair[:, uc, th0:th0 + seg, :]
                        in1 = bass.AP(
                            tensor=qsl.tensor, offset=qsl.offset,
                            ap=[qsl.ap[0], qsl.ap[1], [0, TV // 2], [1, 2]])
                        osl = qk[:, s0:s0 + seg, uc, :]
                        out = bass.AP(
                            tensor=osl.tensor, offset=osl.offset,
                            ap=[osl.ap[0], [CELL, seg], [2, TV // 2], [1, 2]])
                        nc.vector.tensor_tensor(
                            out=out, in0=in0, in1=in1, op=ALU.add)
                    # the previous segment's PSUM drains are emitted AFTER this
                    # segment's adds so the in-order DVE stream never blocks
                    # the next tanh's inputs behind a drain
                    for sc_ps_prev, th_prev in pending:
                        emit_stage(sc_ps_prev, th_prev)
                    pending = []
                    # in-place tanh over this contiguous segment
                    hsl = qk[:, s0:s0 + seg, :, :]
                    hap = bass.AP(
                        tensor=hsl.tensor, offset=hsl.offset,
                        ap=[hsl.ap[0], [1, seg * CELL]])
                    nc.scalar.activation(out=hap, in_=hap, func=AF.Tanh)
                    for ph in range(seg // 8):
                        # 4 t-pairs land on psum partitions {0,32,64,96} of one
                        # bank via column-tiled M=1 matmuls (4 run concurrently
                        # in distinct 32-col groups of the PE array)
                        sc_ps = scpsp.tile([P, 512], fp32, tag="scps")
                        for g in range(4):
                            tp = s0 + ph * 8 + g * 2
                            for uc in range(UC):
                                nc.tensor.matmul(
                                    sc_ps[32 * g:32 * g + 1, :],
                                    scale_f16[:, uc:uc + 1],
                                    qk[:, tp:tp + 2, uc, :],
                                    start=(uc == 0), stop=(uc == UC - 1),
                                    tile_position=(0, 32 * g))
                        pending.append((sc_ps, th0 + ph * 8))
                for sc_ps_prev, th_prev in pending:
                    emit_stage(sc_ps_prev, th_prev)

            # ---------------- softmax + context ---------------------------------
            with tc.tile_pool(name="ep", bufs=1) as eppool, \
                 tc.tile_pool(name="sm", bufs=1) as smpool, \
                 tc.tile_pool(name="epps", bufs=2, space="PSUM") as epps:

                negmax = smpool.tile([P, 1], fp32)
                nc.vector.tensor_reduce(
                    out=negmax, in_=scores_sb, axis=AX.X, op=ALU.max, negate=True)
                attn_sb = smpool.tile([P, TV], fp32)
                ssum = smpool.tile([P, 1], fp32)
                nc.scalar.activation(
                    out=attn_sb, in_=scores_sb, func=AF.Exp, bias=negmax,
                    accum_out=ssum)
                rsum = smpool.tile([P, 1], fp32)
                nc.vector.reciprocal(out=rsum, in_=ssum)
                nc.vector.tensor_scalar_mul(attn_sb, attn_sb, rsum)
                nc.sync.dma_start(out=attn_d[:], in_=attn_sb)

                attn16 = eppool.tile([P, TV], fp16)
                nc.vector.tensor_copy(out=attn16, in_=attn_sb)
                attnT = eppool.tile([P, VC, P], fp16)
                for vc in range(VC):
                    trp = epps.tile([P, P], fp16, tag="trp")
                    nc.tensor.transpose(
                        trp, attn16[:, vc * P:(vc + 1) * P], identity16)
                    nc.any.tensor_copy(out=attnT[:, vc, :], in_=trp)
                ctx_ps = epps.tile([P, D], fp32, tag="ctxps")
                for vc in range(VC):
                    nc.tensor.matmul(
                        ctx_ps,
                        attnT[:, vc, :],
                        v16_sb[:, vc, :],
                        start=(vc == 0), stop=(vc == VC - 1))
                ctx_sb = eppool.tile([P, D], fp32)
                nc.vector.tensor_copy(out=ctx_sb, in_=ctx_ps)
                nc.sync.dma_start(out=ctx_d[:], in_=ctx_sb)

    _split_excess_waits(nc, mybir, cap=1)
    return nc


def _get_nc():
    if "nc" not in _CACHE:
        _CACHE["nc"] = _build()
    return _CACHE["nc"]


def build_in_maps(query, value, mask, W1, W2, scale):
    query = np.asarray(query, dtype=np.float32)
    value = np.asarray(value, dtype=np.float32)
    mask = np.asarray(mask)
    W1 = np.asarray(W1, dtype=np.float32)
    W2 = np.asarray(W2, dtype=np.float32)
    scale = np.asarray(scale, dtype=np.float32)

    sc = np.ascontiguousarray(scale.reshape(UC, P).T)          # [128, 4]
    w1_16 = W1.astype(np.float16)
    w2_16 = W2.astype(np.float16)
    in_maps = []
    for core in range(NCORES):
        b, th = core // 2, core % 2
        amask = ((mask[b].astype(np.float32)) - 1.0) * 1e9
        in_maps.append({
            "qT": np.ascontiguousarray(
                query[b, th * TL:(th + 1) * TL, :].T).astype(np.float16),
            "vT": np.ascontiguousarray(value[b].T).astype(np.float16),
            "v16": np.ascontiguousarray(value[b]).astype(np.float16),
            "w1": w1_16,
            "w2": w2_16,
            "sc": sc,
            "am": np.ascontiguousarray(amask.reshape(1, TV)),
        })
    return in_maps


def kernel(query, value, mask, W1, W2, scale):
    from concourse.bass_utils import run_bass_kernel_spmd

    in_maps = build_in_maps(query, value, mask, W1, W2, scale)
    nc = _get_nc()
    res = run_bass_kernel_spmd(nc, in_maps, list(range(NCORES)))

    context = np.empty((B, TQ, D), dtype=np.float32)
    attn = np.empty((B, TQ, TV), dtype=np.float32)
    for core in range(NCORES):
        b, th = core // 2, core % 2
        context[b, th * TL:(th + 1) * TL, :] = res.results[core]["ctx"]
        attn[b, th * TL:(th + 1) * TL, :] = res.results[core]["attn"]
    return context, attn
